# revision 1
# baseline (speedup 1.0000x reference)
"""Trainium2 Bass kernel for a 3x3 VALID conv2d (dense_cnn).

reference: out[b,o,i,j] = sum_{c,kh,kw} x[b,c,i+kh,j+kw] * w[o,c,kh,kw]
  x: (32, 128, 64, 64) f32, w: (256, 128, 3, 3) f32 -> out: (32, 256, 62, 62) f32

Strategy:
  - Data-parallel over batch: 32 images / 8 cores = 4 images per core;
    weights replicated (pre-transposed on host to [C=128, kh*kw=9, O=256]).
  - Conv = 9 shifted matmuls accumulated in PSUM. Contraction dim C=128 sits
    on the SBUF partition axis.  For an 8-row group of output rows the moving
    operand is x_sb[:, i0+kh : i0+kh+8, kw : kw+62] (N = 8*62 = 496 <= 512)
    and the stationary operand is w_sb[:, kh*3+kw, oc*128:(oc+1)*128].
  - fp32 data is bitcast to float32r for the matmul (full-rate fp32 mode at
    moving free dim >= 256).
"""

import numpy as np

import concourse.bass as bass
import concourse.bacc as bacc
import concourse.mybir as mybir
import concourse.tile as tile

N_CORES = 8
B, C, H, W = 32, 128, 64, 64
O, KH, KW = 256, 3, 3
OH, OW = H - KH + 1, W - KW + 1  # 62, 62
B_LOC = B // N_CORES  # 4
ROWS_PER_GROUP = 8
F32 = mybir.dt.float32
F32R = mybir.dt.float32r

_CACHE: dict = {}


def _build_program() -> bass.Bass:
    nc = bacc.Bacc("TRN2", target_bir_lowering=False, debug=False)

    x_d = nc.dram_tensor("x", [B_LOC, C, H, W], F32R, kind="ExternalInput")
    w_d = nc.dram_tensor("wt", [C, KH * KW, O], F32R, kind="ExternalInput")
    o_d = nc.dram_tensor("out", [B_LOC, O, OH, OW], F32, kind="ExternalOutput")
    x_ap, w_ap, o_ap = x_d.ap(), w_d.ap(), o_d.ap()

    groups = [(i0, min(ROWS_PER_GROUP, OH - i0)) for i0 in range(0, OH, ROWS_PER_GROUP)]

    with tile.TileContext(nc) as tc:
        with (
            tc.tile_pool(name="wpool", bufs=1) as wpool,
            tc.tile_pool(name="xpool", bufs=3) as xpool,
            tc.tile_pool(name="opool", bufs=6) as opool,
            tc.tile_pool(name="warm", bufs=1) as warm,
            tc.tile_pool(name="pspool", bufs=7, space="PSUM") as pspool,
            tc.tile_pool(name="pswarm", bufs=1, space="PSUM") as pswarm,
        ):
            # --- PE clock warm-up: dummy matmuls on a zeroed tile keep the
            # PE busy while the first input DMAs stream in, so the real
            # matmuls run at the full (ramped) clock from the start.
            wz = warm.tile([C, 128], F32R)
            nc.vector.memset(wz.bitcast(F32), 0.0)
            psw = pswarm.tile([128, 128], F32)
            for _ in range(12):
                nc.tensor.matmul(
                    psw, lhsT=wz, rhs=wz, start=True, stop=True
                )

            # --- input loads: first-needed-first, alternating the two HWDGE
            # issuing engines (SP via nc.sync, ACT via nc.scalar) so
            # descriptor generation isn't serialized on one sequencer.
            w_sb = wpool.tile([C, KH * KW, O], F32R)
            x_sbs = []
            for b in range(B_LOC):
                x_sbs.append(xpool.tile([C, H, W], F32R, name="x_sb", tag="x_sb"))

            issue = 0

            def in_dma(out_ap_, in_ap_):
                nonlocal issue
                eng = nc.scalar if issue % 2 == 0 else nc.sync
                eng.dma_start(out=out_ap_, in_=in_ap_)
                issue += 1

            # k=0 weights and image-0 rows 0..15 first (the first row-group's
            # working set), then the remaining weights and image-0 rows.
            in_dma(w_sb[:, 0:1, 0:128], w_ap[:, 0:1, 0:128])
            # rows 0..9 in one chunk: exactly the first row-group's x needs
            in_dma(x_sbs[0][:, 0:10, :], x_ap[0, :, 0:10, :])
            in_dma(w_sb[:, 1:3, 0:128], w_ap[:, 1:3, 0:128])
            in_dma(w_sb[:, 3:6, 0:128], w_ap[:, 3:6, 0:128])
            in_dma(w_sb[:, 6:9, 0:128], w_ap[:, 6:9, 0:128])
            for r0 in range(10, H, 8):
                r1 = min(r0 + 8, H)
                in_dma(x_sbs[0][:, r0:r1, :], x_ap[0, :, r0:r1, :])
            # second output-channel half of the weights: only needed once the
            # oc=1 pass starts, well after the oc=0 groups are underway
            in_dma(w_sb[:, 0:3, 128:256], w_ap[:, 0:3, 128:256])
            in_dma(w_sb[:, 3:6, 128:256], w_ap[:, 3:6, 128:256])
            in_dma(w_sb[:, 6:9, 128:256], w_ap[:, 6:9, 128:256])

            for b in range(B_LOC):
                x_sb = x_sbs[b]
                if b + 1 < B_LOC:
                    # prefetch next image while computing this one
                    for r0 in range(0, H, 16):
                        in_dma(
                            x_sbs[b + 1][:, r0 : r0 + 16, :],
                            x_ap[b + 1, :, r0 : r0 + 16, :],
                        )

                for oc in range(O // 128):
                    for i0, rows in groups:
                        ps = pspool.tile([128, rows, OW], F32)
                        for k in range(KH * KW):
                            kh, kw = divmod(k, KW)
                            nc.tensor.matmul(
                                ps,
                                lhsT=w_sb[:, k, oc * 128 : (oc + 1) * 128],
                                rhs=x_sb[:, i0 + kh : i0 + kh + rows, kw : kw + OW],
                                start=(k == 0),
                                stop=(k == KH * KW - 1),
                            )
                        o_sb = opool.tile([128, rows, OW], F32)
                        nc.vector.tensor_copy(out=o_sb, in_=ps)
                        nc.sync.dma_start(
                            out=o_ap[b, oc * 128 : (oc + 1) * 128, i0 : i0 + rows, :],
                            in_=o_sb,
                        )
    nc.compile()
    return nc


def _get_executor():
    """Build the Bass program once and wrap it in a cached jitted SPMD
    executor (the multi-core path of bass2jax.run_bass_via_pjrt, but with the
    jit object reused across calls so repeated invocations skip recompile)."""
    if "exec" in _CACHE:
        return _CACHE["exec"]

    import jax
    from jax.sharding import Mesh, PartitionSpec
    from jax.experimental.shard_map import shard_map

    from concourse import bass2jax as b2j

    nc = _build_program()
    b2j.install_neuronx_cc_hook()

    partition_name = nc.partition_id_tensor.name if nc.partition_id_tensor else None
    in_names: list[str] = []
    out_names: list[str] = []
    out_avals = []
    for alloc in nc.m.functions[0].allocations:
        if not isinstance(alloc, mybir.MemoryLocationSet):
            continue
        name = alloc.memorylocations[0].name
        if alloc.kind == "ExternalInput":
            if name != partition_name:
                in_names.append(name)
        elif alloc.kind == "ExternalOutput":
            shape = tuple(alloc.tensor_shape)
            dtype = mybir.dt.np(alloc.dtype)
            out_names.append(name)
            out_avals.append(jax.core.ShapedArray(shape, dtype))
    n_params = len(in_names)
    n_outs = len(out_avals)
    all_in_names = in_names + out_names
    if partition_name is not None:
        all_in_names.append(partition_name)
    donate = tuple(range(n_params, n_params + n_outs))

    def _body(*args):
        operands = list(args)
        if partition_name is not None:
            operands.append(b2j.partition_id_tensor())
        outs = b2j._bass_exec_p.bind(
            *operands,
            out_avals=tuple(out_avals),
            in_names=tuple(all_in_names),
            out_names=tuple(out_names),
            lowering_input_output_aliases=(),
            sim_require_finite=True,
            sim_require_nnan=True,
            nc=nc,
        )
        return tuple(outs)

    devices = jax.devices()[:N_CORES]
    mesh = Mesh(np.asarray(devices), ("core",))
    in_specs = (PartitionSpec("core"),) * (n_params + n_outs)
    out_specs = (PartitionSpec("core"),) * n_outs
    sharded = jax.jit(
        shard_map(_body, mesh=mesh, in_specs=in_specs, out_specs=out_specs,
                  check_rep=False),
        donate_argnums=donate,
        keep_unused=True,
    )

    zero_out_shapes = [
        ((N_CORES * a.shape[0], *a.shape[1:]), a.dtype) for a in out_avals
    ]

    def run(in_maps: list[dict[str, np.ndarray]]) -> list[dict[str, np.ndarray]]:
        concat_in = [
            np.concatenate([np.asarray(m[name]) for m in in_maps], axis=0)
            for name in in_names
        ]
        concat_zeros = [np.zeros(s, d) for s, d in zero_out_shapes]
        out_arrs = sharded(*concat_in, *concat_zeros)
        return [
            {
                name: np.asarray(out_arrs[i]).reshape(N_CORES, *out_avals[i].shape)[c]
                for i, name in enumerate(out_names)
            }
            for c in range(N_CORES)
        ]

    _CACHE["exec"] = run
    _CACHE["nc"] = nc
    return run


def kernel(x: np.ndarray, weights: np.ndarray) -> np.ndarray:
    x = np.ascontiguousarray(x, dtype=np.float32)
    # [o, c, kh, kw] -> [c, kh, kw, o] -> [c, kh*kw, o]
    wt = np.ascontiguousarray(
        np.asarray(weights, dtype=np.float32).transpose(1, 2, 3, 0).reshape(C, KH * KW, O)
    )
    run = _get_executor()
    in_maps = [
        {"x": x[i * B_LOC : (i + 1) * B_LOC], "wt": wt} for i in range(N_CORES)
    ]
    results = run(in_maps)
    return np.concatenate([r["out"] for r in results], axis=0)



# revision 4
# speedup vs baseline: 1.0438x; 1.0438x over previous
"""Trainium2 Bass kernel for a 3x3 VALID conv2d (dense_cnn).

reference: out[b,o,i,j] = sum_{c,kh,kw} x[b,c,i+kh,j+kw] * w[o,c,kh,kw]
  x: (32, 128, 64, 64) f32, w: (256, 128, 3, 3) f32 -> out: (32, 256, 62, 62) f32

Strategy:
  - Data-parallel over batch: 32 images / 8 cores = 4 images per core;
    weights replicated.
  - fp8(e4m3) DoubleRow matmuls: one DR matmul contracts TWO K=128 planes
    (result = wA^T@xA + wB^T@xB) at 0.5 PE cycles per output element --
    2x the MAC rate of full-rate fp32r.
  - Precision recovered with a hi/lo split: x ~ x_hi + x_lo and
    w ~ w8 + w_lo (each term e4m3).  Per output row we compute 26 of the
    27 first-order products (drop x-corr on tap 8), paired into 13 DR
    matmuls:
      m=0..7 : halves (w8[k] (x) x_hi[k],  w8[k]  (x) x_lo[k])   k=m
      m=8    : halves (w8[8] (x) x_hi[8],  wlo[8] (x) x_hi[8])
      m=9..12: halves (wlo[a] (x) x_hi[a], wlo[b] (x) x_hi[b]),
               (a,b) in (0,1),(2,3),(4,5),(6,7)
    -> rel_l2 ~ 7e-3 (vs 2e-2 gate), PE cost 13/18 of the fp32r direct
    conv (83us vs 115us of PE time).
  - PSUM: one 2KB bank holds 8 output rows (62 cols padded to 64 = 256B
    per row); start only on the bank's first matmul, stop on its last
    (pending-zero gives each row's first touch overwrite semantics).
  - hi/lo pairs sit in one SBUF tile [128, 2, 64, 64] so DR ifmap APs are
    natural slices; tap-pair APs use a custom pair-dim stride.
"""

import numpy as np
import ml_dtypes

import bass_rust
import concourse.bass as bass
import concourse.bacc as bacc
import concourse.mybir as mybir
import concourse.tile as tile

N_CORES = 8
B, C, H, W = 32, 128, 64, 64
O, KH, KW = 256, 3, 3
OH, OW = H - KH + 1, W - KW + 1  # 62, 62
B_LOC = B // N_CORES  # 4
ROWS_PER_BANK = 8
F8 = mybir.dt.float8e4
F32 = mybir.dt.float32
E4 = ml_dtypes.float8_e4m3
DR = mybir.MatmulPerfMode.DoubleRow

# DR matmul plan: 13 entries per output row.
#   kind "hilo": rhs halves (x_hi[tap], x_lo[tap]) -- natural plane-pair slice
#   kind "bcast": rhs halves (x_hi[tap], x_hi[tap]) -- stride-0 pair dim
#   kind "pair": rhs halves (x_hi[tap_a], x_hi[tap_b]) -- tap_b is read from a
#     pre-shifted copy of the hi plane (plane 2: +1 col, plane 3: +1 row/-2
#     col), so the pair stride is a whole number of 4KB planes.  (Pair strides
#     smaller than the row span crash the PE's DoubleRow ifmap fetch.)
TAPS = [(k // 3, k % 3) for k in range(9)]
PLAN = (
    [("hilo", k, None) for k in range(8)]
    + [("bcast", 8, None)]
    + [("pair", a, b) for a, b in ((0, 1), (2, 3), (4, 5), (6, 7))]
)
PLANE_SZ = 64 * 64  # elements per [H, W] plane

_CACHE: dict = {}


def _build_program() -> bass.Bass:
    nc = bacc.Bacc("TRN2", target_bir_lowering=False, debug=False)

    # x hi/lo packed: [b][c][half][h][w] fp8
    x_d = nc.dram_tensor("x8", [B_LOC, C, 2, H, W], F8, kind="ExternalInput")
    # packed DR weights: [c][oc_half][m][pair_half][o_local] fp8
    w_d = nc.dram_tensor("wpk", [C, 2, 13, 2, 128], F8, kind="ExternalInput")
    o_d = nc.dram_tensor("out", [B_LOC, O, OH, OW], F32, kind="ExternalOutput")
    x_ap, w_ap, o_ap = x_d.ap(), w_d.ap(), o_d.ap()

    groups = [(i0, min(ROWS_PER_BANK, OH - i0)) for i0 in range(0, OH, ROWS_PER_BANK)]

    with tile.TileContext(nc) as tc:
        with (
            tc.tile_pool(name="wpool", bufs=1) as wpool,
            tc.tile_pool(name="xpool", bufs=1) as xpool,
            tc.tile_pool(name="opool", bufs=6) as opool,
            tc.tile_pool(name="warm", bufs=1) as warm,
            tc.tile_pool(name="pspool", bufs=7, space="PSUM") as pspool,
            tc.tile_pool(name="pswarm", bufs=1, space="PSUM") as pswarm,
        ):
            # --- PE clock warm-up while input DMAs stream in.
            wz = warm.tile([C, 256], F8)
            nc.vector.memset(wz, 0.0)
            psw = pswarm.tile([128, 512], F32)
            for _ in range(24):
                nc.tensor.matmul(
                    psw[:, 0:128], lhsT=wz[:, 0:128], rhs=wz[:, 0:128],
                    start=True, stop=True,
                )

            # --- input loads, first-needed-first, on both HWDGE queues
            # x planes: 0 = hi, 1 = lo, 2 = hi shifted +1 col,
            #           3 = hi shifted (+1 row, -2 col)
            w_sb = wpool.tile([C, 2, 13, 2, 128], F8)
            x_sbs = [xpool.tile([C, 4, H, W], F8, name=f"x_sb{b}") for b in range(B_LOC)]

            issue = 0

            def in_dma(out_ap_, in_ap_):
                nonlocal issue
                eng = nc.scalar if issue % 2 == 0 else nc.sync
                eng.dma_start(out=out_ap_, in_=in_ap_)
                issue += 1

            def x_dmas(b, chunks):
                for r0, r1 in chunks:
                    in_dma(x_sbs[b][:, 0:2, r0:r1, :], x_ap[b][:, :, r0:r1, :])
                    in_dma(
                        x_sbs[b][:, 2, r0:r1, 0 : W - 1],
                        x_ap[b][:, 0, r0:r1, 1:W],
                    )
                    r1c = min(r1, H - 1)
                    in_dma(
                        x_sbs[b][:, 3, r0:r1c, 2:W],
                        x_ap[b][:, 0, r0 + 1 : r1c + 1, 0 : W - 2],
                    )

            in_dma(w_sb, w_ap)
            x_dmas(0, [(0, 16), (16, 64)])
            for b in range(1, B_LOC):
                x_dmas(b, [(0, 64)])

            def rhs_for(x_sb, m, r):
                kind, a, bb = PLAN[m]
                kh, kw = TAPS[a]
                if kind == "hilo":
                    return x_sb[:, 0:2, r + kh, kw : kw + OW]
                base = x_sb[:, 0, r + kh, kw : kw + OW]
                if kind == "bcast":
                    return base.unsqueeze(1).broadcast_to((C, 2, OW))
                # cross-tap pair via shifted plane: plane 3 for the
                # (0,2)->(1,0) pair, plane 2 for the +1-col pairs
                nplanes = 3 if bb == 3 else 2
                rhs = base.copy()
                part = tuple(rhs.ap[0])
                rhs.ap = bass_rust.VecI64Pair(
                    [part, (nplanes * PLANE_SZ, 2), (1, OW)]
                )
                return rhs

            drain = 0
            for b in range(B_LOC):
                x_sb = x_sbs[b]
                for oc in range(2):
                    lhs = [w_sb[:, oc, m, :, :] for m in range(13)]
                    for i0, rows in groups:
                        ps = pspool.tile([128, ROWS_PER_BANK, 64], F32)
                        for r_loc in range(rows):
                            r = i0 + r_loc
                            for m in range(13):
                                nc.tensor.matmul(
                                    ps[:, r_loc, 0:OW],
                                    lhsT=lhs[m],
                                    rhs=rhs_for(x_sb, m, r),
                                    start=(r_loc == 0 and m == 0),
                                    stop=(r_loc == rows - 1 and m == 12),
                                    perf_mode=DR,
                                )
                        o_sb = opool.tile([128, ROWS_PER_BANK, OW], F32)
                        src = ps[:, 0:rows, 0:OW]
                        dst = o_sb[:, 0:rows, :]
                        if drain % 2 == 0:
                            nc.vector.tensor_copy(out=dst, in_=src)
                        else:
                            nc.scalar.copy(out=dst, in_=src)
                        eng = nc.sync if drain % 2 == 0 else nc.scalar
                        eng.dma_start(
                            out=o_ap[b, oc * 128 : (oc + 1) * 128, i0 : i0 + rows, :],
                            in_=dst,
                        )
                        drain += 1
    nc.compile()
    return nc


def _get_executor():
    """Build the Bass program once and wrap it in a cached jitted SPMD
    executor (the multi-core path of bass2jax.run_bass_via_pjrt, but with the
    jit object reused across calls so repeated invocations skip recompile)."""
    if "exec" in _CACHE:
        return _CACHE["exec"]

    import jax
    from jax.sharding import Mesh, PartitionSpec
    from jax.experimental.shard_map import shard_map

    from concourse import bass2jax as b2j

    nc = _build_program()
    b2j.install_neuronx_cc_hook()

    partition_name = nc.partition_id_tensor.name if nc.partition_id_tensor else None
    in_names: list[str] = []
    out_names: list[str] = []
    out_avals = []
    for alloc in nc.m.functions[0].allocations:
        if not isinstance(alloc, mybir.MemoryLocationSet):
            continue
        name = alloc.memorylocations[0].name
        if alloc.kind == "ExternalInput":
            if name != partition_name:
                in_names.append(name)
        elif alloc.kind == "ExternalOutput":
            shape = tuple(alloc.tensor_shape)
            dtype = mybir.dt.np(alloc.dtype)
            out_names.append(name)
            out_avals.append(jax.core.ShapedArray(shape, dtype))
    n_params = len(in_names)
    n_outs = len(out_avals)
    all_in_names = in_names + out_names
    if partition_name is not None:
        all_in_names.append(partition_name)
    donate = tuple(range(n_params, n_params + n_outs))

    def _body(*args):
        operands = list(args)
        if partition_name is not None:
            operands.append(b2j.partition_id_tensor())
        outs = b2j._bass_exec_p.bind(
            *operands,
            out_avals=tuple(out_avals),
            in_names=tuple(all_in_names),
            out_names=tuple(out_names),
            lowering_input_output_aliases=(),
            sim_require_finite=True,
            sim_require_nnan=True,
            nc=nc,
        )
        return tuple(outs)

    devices = jax.devices()[:N_CORES]
    mesh = Mesh(np.asarray(devices), ("core",))
    in_specs = (PartitionSpec("core"),) * (n_params + n_outs)
    out_specs = (PartitionSpec("core"),) * n_outs
    sharded = jax.jit(
        shard_map(_body, mesh=mesh, in_specs=in_specs, out_specs=out_specs,
                  check_rep=False),
        donate_argnums=donate,
        keep_unused=True,
    )

    zero_out_shapes = [
        ((N_CORES * a.shape[0], *a.shape[1:]), a.dtype) for a in out_avals
    ]

    def run(in_maps: list[dict[str, np.ndarray]]) -> list[dict[str, np.ndarray]]:
        concat_in = [
            np.concatenate([np.asarray(m[name]) for m in in_maps], axis=0)
            for name in in_names
        ]
        concat_zeros = [np.zeros(s, d) for s, d in zero_out_shapes]
        out_arrs = sharded(*concat_in, *concat_zeros)
        return [
            {
                name: np.asarray(out_arrs[i]).reshape(N_CORES, *out_avals[i].shape)[c]
                for i, name in enumerate(out_names)
            }
            for c in range(N_CORES)
        ]

    _CACHE["exec"] = run
    _CACHE["nc"] = nc
    return run


def _pack_weights(weights: np.ndarray) -> np.ndarray:
    """[o, c, kh, kw] f32 -> packed DR plan [c, oc_half, m, pair_half, o_local] fp8."""
    wf = np.asarray(weights, dtype=np.float32)
    w8 = wf.astype(E4)
    wlo = (wf - w8.astype(np.float32)).astype(E4)
    # [o, c, k] -> [c, k, o]
    w8t = w8.reshape(O, C, 9).transpose(1, 2, 0)
    wlot = wlo.reshape(O, C, 9).transpose(1, 2, 0)
    wpk = np.zeros((C, 2, 13, 2, 128), dtype=E4)
    for oc in range(2):
        sl = slice(oc * 128, (oc + 1) * 128)
        for m, (kind, a, bb) in enumerate(PLAN):
            if kind == "hilo":
                wpk[:, oc, m, 0, :] = w8t[:, a, sl]
                wpk[:, oc, m, 1, :] = w8t[:, a, sl]
            elif kind == "bcast":
                wpk[:, oc, m, 0, :] = w8t[:, a, sl]
                wpk[:, oc, m, 1, :] = wlot[:, a, sl]
            else:
                wpk[:, oc, m, 0, :] = wlot[:, a, sl]
                wpk[:, oc, m, 1, :] = wlot[:, bb, sl]
    return wpk


def kernel(x: np.ndarray, weights: np.ndarray) -> np.ndarray:
    xf = np.ascontiguousarray(x, dtype=np.float32)
    x_hi = xf.astype(E4)
    x_lo = (xf - x_hi.astype(np.float32)).astype(E4)
    # [b, c, 2, h, w]
    x8 = np.ascontiguousarray(np.stack([x_hi, x_lo], axis=2))
    wpk = _pack_weights(weights)

    run = _get_executor()
    in_maps = [
        {"x8": x8[i * B_LOC : (i + 1) * B_LOC], "wpk": wpk} for i in range(N_CORES)
    ]
    results = run(in_maps)
    return np.concatenate([r["out"] for r in results], axis=0)


# revision 5
# speedup vs baseline: 1.2412x; 1.1891x over previous
"""Trainium2 Bass kernel for a 3x3 VALID conv2d (dense_cnn).

reference: out[b,o,i,j] = sum_{c,kh,kw} x[b,c,i+kh,j+kw] * w[o,c,kh,kw]
  x: (32, 128, 64, 64) f32, w: (256, 128, 3, 3) f32 -> out: (32, 256, 62, 62) f32

Strategy:
  - Data-parallel over batch: 32 images / 8 cores = 4 images per core;
    weights replicated.
  - fp8(e4m3) DoubleRow matmuls: one DR matmul contracts TWO K=128 planes
    (result = wA^T@xA + wB^T@xB) at 0.5 PE cycles per output element --
    2x the MAC rate of full-rate fp32r.
  - Precision recovered with a hi/lo split: x ~ x_hi + x_lo and
    w ~ w8 + w_lo (each term e4m3).  Per output row we compute 26 of the
    27 first-order products (dropping the x-correction on tap 8), paired
    into 13 DR matmuls:
      m=0..7 : halves (w8[k] (x) x_hi[k],  w8[k]  (x) x_lo[k])   k=m
      m=8    : halves (w8[8] (x) x_hi[8],  wlo[8] (x) x_hi[8])
      m=9..11: halves (wlo[a] (x) x_hi[a], wlo[a+3] (x) x_hi[a+3]), a=0,1,2
      m=12   : halves (wlo[6] (x) x_hi[6], wlo[7] (x) x_hi[7])
    -> rel_l2 ~ 9e-3 (vs 2e-2 gate), PE cost 13/18 of the fp32r direct
    conv (83us vs 115us of PE busy time).
  - DR ifmap pair strides must be >= the row span: hi/lo pairs use the
    plane stride (4096), row pairs (a, a+3) use stride 64, and the one
    column pair (6,7) reads a 1-col-shifted copy of the hi plane (built by
    the otherwise-idle gpsimd engine) at stride 2*4096.
  - PSUM: one 2KB bank holds 8 output rows (62 cols padded to 64 = 256B
    per row); start only on the bank's first matmul, stop on its last
    (pending-zero gives each row's first touch overwrite semantics).
  - Output is drained PSUM->SBUF as bf16 (alternating DVE / Act engines),
    DMA'd out as bf16 to halve output HBM traffic, and upcast to f32 on
    the host.  Input DMAs for image b are issued at the start of image
    b-1's compute so input and output transfers share the DMA fabric
    evenly.
"""

import numpy as np
import ml_dtypes

import bass_rust
import concourse.bass as bass
import concourse.bacc as bacc
import concourse.mybir as mybir
import concourse.tile as tile

N_CORES = 8
B, C, H, W = 32, 128, 64, 64
O, KH, KW = 256, 3, 3
OH, OW = H - KH + 1, W - KW + 1  # 62, 62
B_LOC = B // N_CORES  # 4
ROWS_PER_BANK = 8
F8 = mybir.dt.float8e4
F32 = mybir.dt.float32
BF16 = mybir.dt.bfloat16
E4 = ml_dtypes.float8_e4m3
DR = mybir.MatmulPerfMode.DoubleRow

TAPS = [(k // 3, k % 3) for k in range(9)]
PLAN = (
    [("hilo", k, None) for k in range(8)]
    + [("bcast", 8, None)]
    + [("rowpair", a, a + 3) for a in range(3)]
    + [("p2pair", 6, 7)]
)
PLANE_SZ = H * W  # elements per [H, W] plane

_CACHE: dict = {}


def _build_program() -> bass.Bass:
    nc = bacc.Bacc("TRN2", target_bir_lowering=False, debug=False)

    # x hi/lo packed: [b][c][half][h][w] fp8
    x_d = nc.dram_tensor("x8", [B_LOC, C, 2, H, W], F8, kind="ExternalInput")
    # packed DR weights: [c][oc_half][m][pair_half][o_local] fp8
    w_d = nc.dram_tensor("wpk", [C, 2, 13, 2, 128], F8, kind="ExternalInput")
    o_d = nc.dram_tensor("out", [B_LOC, O, OH, OW], BF16, kind="ExternalOutput")
    x_ap, w_ap, o_ap = x_d.ap(), w_d.ap(), o_d.ap()

    groups = [(i0, min(ROWS_PER_BANK, OH - i0)) for i0 in range(0, OH, ROWS_PER_BANK)]

    with tile.TileContext(nc) as tc:
        with (
            tc.tile_pool(name="wpool", bufs=1) as wpool,
            tc.tile_pool(name="xpool", bufs=1) as xpool,
            tc.tile_pool(name="opool", bufs=6) as opool,
            tc.tile_pool(name="warm", bufs=1) as warm,
            tc.tile_pool(name="pspool", bufs=7, space="PSUM") as pspool,
            tc.tile_pool(name="pswarm", bufs=1, space="PSUM") as pswarm,
        ):
            # --- PE clock warm-up while the first input DMAs stream in.
            wz = warm.tile([C, 256], F8)
            nc.vector.memset(wz, 0.0)
            psw = pswarm.tile([128, 512], F32)
            for _ in range(28):
                nc.tensor.matmul(
                    psw[:, 0:128], lhsT=wz[:, 0:128], rhs=wz[:, 0:128],
                    start=True, stop=True,
                )

            # x planes: 0 = hi, 1 = lo, 2 = hi shifted +1 col (gpsimd copy)
            w_sb = wpool.tile([C, 2, 13, 2, 128], F8)
            x_sbs = [xpool.tile([C, 3, H, W], F8, name=f"x_sb{b}") for b in range(B_LOC)]

            issue = 0

            def in_dma(out_ap_, in_ap_):
                nonlocal issue
                eng = nc.scalar if issue % 2 == 0 else nc.sync
                eng.dma_start(out=out_ap_, in_=in_ap_)
                issue += 1

            def x_load(b, chunks):
                x_sb = x_sbs[b]
                for r0, r1 in chunks:
                    in_dma(x_sb[:, 0:2, r0:r1, :], x_ap[b][:, :, r0:r1, :])
                for r0, r1 in chunks:
                    # plane 2 rows [max(r0,2), r1): hi[h, 1:64] -> [h, 0:63]
                    p0 = max(r0, 2)
                    nc.gpsimd.tensor_copy(
                        out=x_sb[:, 2, p0:r1, 0 : W - 1],
                        in_=x_sb[:, 0, p0:r1, 1:W],
                    )

            in_dma(w_sb, w_ap)
            x_load(0, [(0, 16), (16, 40), (40, 64)])

            def rhs_for(x_sb, m, r):
                kind, a, bb = PLAN[m]
                kh, kw = TAPS[a]
                base = x_sb[:, 0, r + kh, kw : kw + OW]
                if kind == "hilo":
                    return x_sb[:, 0:2, r + kh, kw : kw + OW]
                if kind == "bcast":
                    return base.unsqueeze(1).broadcast_to((C, 2, OW))
                delta = W if kind == "rowpair" else 2 * PLANE_SZ
                rhs = base.copy()
                part = tuple(rhs.ap[0])
                rhs.ap = bass_rust.VecI64Pair([part, (delta, 2), (1, OW)])
                return rhs

            drain = 0
            for b in range(B_LOC):
                if b + 1 < B_LOC:
                    x_load(b + 1, [(0, 32), (32, 64)])
                x_sb = x_sbs[b]
                for oc in range(2):
                    lhs = [w_sb[:, oc, m, :, :] for m in range(13)]
                    for i0, rows in groups:
                        ps = pspool.tile([128, ROWS_PER_BANK, 64], F32)
                        for r_loc in range(rows):
                            r = i0 + r_loc
                            for m in range(13):
                                nc.tensor.matmul(
                                    ps[:, r_loc, 0:OW],
                                    lhsT=lhs[m],
                                    rhs=rhs_for(x_sb, m, r),
                                    start=(r_loc == 0 and m == 0),
                                    stop=(r_loc == rows - 1 and m == 12),
                                    perf_mode=DR,
                                )
                        o_sb = opool.tile([128, ROWS_PER_BANK, OW], BF16)
                        src = ps[:, 0:rows, 0:OW]
                        dst = o_sb[:, 0:rows, :]
                        if drain % 2 == 0:
                            nc.vector.tensor_copy(out=dst, in_=src)
                        else:
                            nc.scalar.copy(out=dst, in_=src)
                        eng = nc.sync if drain % 2 == 0 else nc.scalar
                        eng.dma_start(
                            out=o_ap[b, oc * 128 : (oc + 1) * 128, i0 : i0 + rows, :],
                            in_=dst,
                        )
                        drain += 1
    nc.compile()
    return nc


def _get_executor():
    """Build the Bass program once and wrap it in a cached jitted SPMD
    executor (the multi-core path of bass2jax.run_bass_via_pjrt, but with the
    jit object reused across calls so repeated invocations skip recompile)."""
    if "exec" in _CACHE:
        return _CACHE["exec"]

    import jax
    from jax.sharding import Mesh, PartitionSpec
    from jax.experimental.shard_map import shard_map

    from concourse import bass2jax as b2j

    nc = _build_program()
    b2j.install_neuronx_cc_hook()

    partition_name = nc.partition_id_tensor.name if nc.partition_id_tensor else None
    in_names: list[str] = []
    out_names: list[str] = []
    out_avals = []
    for alloc in nc.m.functions[0].allocations:
        if not isinstance(alloc, mybir.MemoryLocationSet):
            continue
        name = alloc.memorylocations[0].name
        if alloc.kind == "ExternalInput":
            if name != partition_name:
                in_names.append(name)
        elif alloc.kind == "ExternalOutput":
            shape = tuple(alloc.tensor_shape)
            dtype = mybir.dt.np(alloc.dtype)
            out_names.append(name)
            out_avals.append(jax.core.ShapedArray(shape, dtype))
    n_params = len(in_names)
    n_outs = len(out_avals)
    all_in_names = in_names + out_names
    if partition_name is not None:
        all_in_names.append(partition_name)
    donate = tuple(range(n_params, n_params + n_outs))

    def _body(*args):
        operands = list(args)
        if partition_name is not None:
            operands.append(b2j.partition_id_tensor())
        outs = b2j._bass_exec_p.bind(
            *operands,
            out_avals=tuple(out_avals),
            in_names=tuple(all_in_names),
            out_names=tuple(out_names),
            lowering_input_output_aliases=(),
            sim_require_finite=True,
            sim_require_nnan=True,
            nc=nc,
        )
        return tuple(outs)

    devices = jax.devices()[:N_CORES]
    mesh = Mesh(np.asarray(devices), ("core",))
    in_specs = (PartitionSpec("core"),) * (n_params + n_outs)
    out_specs = (PartitionSpec("core"),) * n_outs
    sharded = jax.jit(
        shard_map(_body, mesh=mesh, in_specs=in_specs, out_specs=out_specs,
                  check_rep=False),
        donate_argnums=donate,
        keep_unused=True,
    )

    zero_out_shapes = [
        ((N_CORES * a.shape[0], *a.shape[1:]), a.dtype) for a in out_avals
    ]

    def run(in_maps: list[dict[str, np.ndarray]]) -> list[dict[str, np.ndarray]]:
        concat_in = [
            np.concatenate([np.asarray(m[name]) for m in in_maps], axis=0)
            for name in in_names
        ]
        concat_zeros = [np.zeros(s, d) for s, d in zero_out_shapes]
        out_arrs = sharded(*concat_in, *concat_zeros)
        return [
            {
                name: np.asarray(out_arrs[i]).reshape(N_CORES, *out_avals[i].shape)[c]
                for i, name in enumerate(out_names)
            }
            for c in range(N_CORES)
        ]

    _CACHE["exec"] = run
    _CACHE["nc"] = nc
    return run


def _pack_weights(weights: np.ndarray) -> np.ndarray:
    """[o, c, kh, kw] f32 -> packed DR plan [c, oc_half, m, pair_half, o_local] fp8."""
    wf = np.asarray(weights, dtype=np.float32)
    w8 = wf.astype(E4)
    wlo = (wf - w8.astype(np.float32)).astype(E4)
    # [o, c, k] -> [c, k, o]
    w8t = w8.reshape(O, C, 9).transpose(1, 2, 0)
    wlot = wlo.reshape(O, C, 9).transpose(1, 2, 0)
    wpk = np.zeros((C, 2, 13, 2, 128), dtype=E4)
    for oc in range(2):
        sl = slice(oc * 128, (oc + 1) * 128)
        for m, (kind, a, bb) in enumerate(PLAN):
            if kind == "hilo":
                wpk[:, oc, m, 0, :] = w8t[:, a, sl]
                wpk[:, oc, m, 1, :] = w8t[:, a, sl]
            elif kind == "bcast":
                wpk[:, oc, m, 0, :] = w8t[:, a, sl]
                wpk[:, oc, m, 1, :] = wlot[:, a, sl]
            else:
                wpk[:, oc, m, 0, :] = wlot[:, a, sl]
                wpk[:, oc, m, 1, :] = wlot[:, bb, sl]
    return wpk


def kernel(x: np.ndarray, weights: np.ndarray) -> np.ndarray:
    xf = np.ascontiguousarray(x, dtype=np.float32)
    x_hi = xf.astype(E4)
    x_lo = (xf - x_hi.astype(np.float32)).astype(E4)
    # [b, c, 2, h, w]
    x8 = np.ascontiguousarray(np.stack([x_hi, x_lo], axis=2))
    wpk = _pack_weights(weights)

    run = _get_executor()
    in_maps = [
        {"x8": x8[i * B_LOC : (i + 1) * B_LOC], "wpk": wpk} for i in range(N_CORES)
    ]
    results = run(in_maps)
    out16 = np.concatenate([r["out"] for r in results], axis=0)
    return out16.astype(np.float32)


# revision 7
# speedup vs baseline: 1.3284x; 1.0703x over previous
"""Trainium2 Bass kernel for a 3x3 VALID conv2d (dense_cnn).

reference: out[b,o,i,j] = sum_{c,kh,kw} x[b,c,i+kh,j+kw] * w[o,c,kh,kw]
  x: (32, 128, 64, 64) f32, w: (256, 128, 3, 3) f32 -> out: (32, 256, 62, 62) f32

Strategy:
  - Data-parallel over batch: 32 images / 8 cores = 4 images per core;
    weights replicated.
  - fp8(e4m3) DoubleRow matmuls: one DR matmul contracts TWO K=128 planes
    (result = wA^T@xA + wB^T@xB) at 0.5 PE cycles per output element --
    2x the MAC rate of full-rate fp32r.
  - Precision recovered with a hi/lo split: x ~ x_hi + x_lo and
    w ~ w8 + w_lo (each term e4m3).  Per output row we compute 26 of the
    27 first-order products (dropping the x-correction on tap 8), paired
    into 13 DR matmuls:
      m=0..7 : halves (w8[k] (x) x_hi[k],  w8[k]  (x) x_lo[k])   k=m
      m=8    : halves (w8[8] (x) x_hi[8],  wlo[8] (x) x_hi[8])
      m=9..11: halves (wlo[a] (x) x_hi[a], wlo[a+3] (x) x_hi[a+3]), a=0,1,2
      m=12   : halves (wlo[6] (x) x_hi[6], wlo[7] (x) x_hi[7])
    -> rel_l2 ~ 9e-3 (vs 2e-2 gate), PE cost 13/18 of the fp32r direct
    conv (83us vs 115us of PE busy time).
  - DR ifmap pair strides must be >= the row span: hi/lo pairs use the
    plane stride (4096), row pairs (a, a+3) use stride 64, and the one
    column pair (6,7) reads a 1-col-shifted copy of the hi plane (built by
    the otherwise-idle gpsimd engine) at stride 2*4096.
  - PSUM: one 2KB bank holds 8 output rows (62 cols padded to 64 = 256B
    per row); start only on the bank's first matmul, stop on its last
    (pending-zero gives each row's first touch overwrite semantics).
  - Output is drained PSUM->SBUF as bf16 (alternating DVE / Act engines),
    DMA'd out as bf16 to halve output HBM traffic, and upcast to f32 on
    the host.  Input DMAs for image b are issued at the start of image
    b-1's compute so input and output transfers share the DMA fabric
    evenly.
"""

import numpy as np
import ml_dtypes

import bass_rust
import concourse.bass as bass
import concourse.bacc as bacc
import concourse.mybir as mybir
import concourse.tile as tile

N_CORES = 8
B, C, H, W = 32, 128, 64, 64
O, KH, KW = 256, 3, 3
OH, OW = H - KH + 1, W - KW + 1  # 62, 62
B_LOC = B // N_CORES  # 4
ROWS_PER_BANK = 8
F8 = mybir.dt.float8e4
F32 = mybir.dt.float32
BF16 = mybir.dt.bfloat16
E4 = ml_dtypes.float8_e4m3
DR = mybir.MatmulPerfMode.DoubleRow

TAPS = [(k // 3, k % 3) for k in range(9)]
PLAN = (
    [("hilo", k, None) for k in range(8)]
    + [("bcast", 8, None)]
    + [("rowpair", a, a + 3) for a in range(3)]
    + [("p2pair", 6, 7)]
)
PLANE_SZ = H * W  # elements per [H, W] plane

_CACHE: dict = {}


def _build_program() -> bass.Bass:
    nc = bacc.Bacc("TRN2", target_bir_lowering=False, debug=False)

    # x hi/lo packed: [b][c][half][h][w] fp8
    x_d = nc.dram_tensor("x8", [B_LOC, C, 2, H, W], F8, kind="ExternalInput")
    # packed DR weights: [c][oc_half][m][pair_half][o_local] fp8
    w_d = nc.dram_tensor("wpk", [C, 2, 13, 2, 128], F8, kind="ExternalInput")
    o_d = nc.dram_tensor("out", [B_LOC, O, OH, OW], BF16, kind="ExternalOutput")
    x_ap, w_ap, o_ap = x_d.ap(), w_d.ap(), o_d.ap()

    groups = [(i0, min(ROWS_PER_BANK, OH - i0)) for i0 in range(0, OH, ROWS_PER_BANK)]

    with tile.TileContext(nc) as tc:
        with (
            tc.tile_pool(name="wpool", bufs=1) as wpool,
            tc.tile_pool(name="xpool", bufs=1) as xpool,
            tc.tile_pool(name="opool", bufs=6) as opool,
            tc.tile_pool(name="warm", bufs=1) as warm,
            tc.tile_pool(name="pspool", bufs=7, space="PSUM") as pspool,
            tc.tile_pool(name="pswarm", bufs=1, space="PSUM") as pswarm,
        ):
            # --- PE clock warm-up while the first input DMAs stream in.
            wz = warm.tile([C, 256], F8)
            nc.gpsimd.memset(wz, 0.0)
            psw = pswarm.tile([128, 512], F32)
            for _ in range(20):
                nc.tensor.matmul(
                    psw[:, 0:128], lhsT=wz[:, 0:128], rhs=wz[:, 0:128],
                    start=True, stop=True,
                )

            # x planes: 0 = hi, 1 = lo, 2 = hi shifted +1 col (gpsimd copy)
            w_sb = wpool.tile([C, 2, 13, 2, 128], F8)
            x_sbs = [xpool.tile([C, 3, H, W], F8, name=f"x_sb{b}") for b in range(B_LOC)]

            issue = 0

            def in_dma(out_ap_, in_ap_):
                nonlocal issue
                eng = nc.scalar if issue % 2 == 0 else nc.sync
                eng.dma_start(out=out_ap_, in_=in_ap_)
                issue += 1

            def x_load(b, chunks, eng=None):
                x_sb = x_sbs[b]
                for r0, r1 in chunks:
                    if eng is None:
                        in_dma(x_sb[:, 0:2, r0:r1, :], x_ap[b][:, :, r0:r1, :])
                    else:
                        eng.dma_start(
                            out=x_sb[:, 0:2, r0:r1, :], in_=x_ap[b][:, :, r0:r1, :]
                        )
                    # plane 2 rows [max(r0,2), r1): hi[h, 1:64] -> [h, 0:63]
                    p0 = max(r0, 2)
                    nc.gpsimd.tensor_copy(
                        out=x_sb[:, 2, p0:r1, 0 : W - 1],
                        in_=x_sb[:, 0, p0:r1, 1:W],
                    )

            # weights alone on the Act queue; img0's chunks stream on SP so
            # neither blocks the other, with the plane-2 gpsimd copies
            # chasing each landed chunk.
            nc.scalar.dma_start(out=w_sb, in_=w_ap)
            x_load(0, [(0, 16), (16, 32), (32, 48), (48, 64)], eng=nc.sync)

            def rhs_for(x_sb, m, r):
                kind, a, bb = PLAN[m]
                kh, kw = TAPS[a]
                base = x_sb[:, 0, r + kh, kw : kw + OW]
                if kind == "hilo":
                    return x_sb[:, 0:2, r + kh, kw : kw + OW]
                if kind == "bcast":
                    return base.unsqueeze(1).broadcast_to((C, 2, OW))
                delta = W if kind == "rowpair" else 2 * PLANE_SZ
                rhs = base.copy()
                part = tuple(rhs.ap[0])
                rhs.ap = bass_rust.VecI64Pair([part, (delta, 2), (1, OW)])
                return rhs

            drain = 0
            for b in range(B_LOC):
                if b + 1 < B_LOC:
                    x_load(b + 1, [(0, 32), (32, 64)])
                x_sb = x_sbs[b]
                for oc in range(2):
                    lhs = [w_sb[:, oc, m, :, :] for m in range(13)]
                    for i0, rows in groups:
                        ps = pspool.tile([128, ROWS_PER_BANK, 64], F32)
                        for r_loc in range(rows):
                            r = i0 + r_loc
                            for m in range(13):
                                nc.tensor.matmul(
                                    ps[:, r_loc, 0:OW],
                                    lhsT=lhs[m],
                                    rhs=rhs_for(x_sb, m, r),
                                    start=(r_loc == 0 and m == 0),
                                    stop=(r_loc == rows - 1 and m == 12),
                                    perf_mode=DR,
                                )
                        o_sb = opool.tile([128, ROWS_PER_BANK, OW], BF16)
                        src = ps[:, 0:rows, 0:OW]
                        dst = o_sb[:, 0:rows, :]
                        if drain % 2 == 0:
                            nc.vector.tensor_copy(out=dst, in_=src)
                        else:
                            nc.scalar.copy(out=dst, in_=src)
                        eng = nc.sync if drain % 2 == 0 else nc.scalar
                        eng.dma_start(
                            out=o_ap[b, oc * 128 : (oc + 1) * 128, i0 : i0 + rows, :],
                            in_=dst,
                        )
                        drain += 1
    nc.compile()
    return nc


def _get_executor():
    """Build the Bass program once and wrap it in a cached jitted SPMD
    executor (the multi-core path of bass2jax.run_bass_via_pjrt, but with the
    jit object reused across calls so repeated invocations skip recompile)."""
    if "exec" in _CACHE:
        return _CACHE["exec"]

    import jax
    from jax.sharding import Mesh, PartitionSpec
    from jax.experimental.shard_map import shard_map

    from concourse import bass2jax as b2j

    nc = _build_program()
    b2j.install_neuronx_cc_hook()

    partition_name = nc.partition_id_tensor.name if nc.partition_id_tensor else None
    in_names: list[str] = []
    out_names: list[str] = []
    out_avals = []
    for alloc in nc.m.functions[0].allocations:
        if not isinstance(alloc, mybir.MemoryLocationSet):
            continue
        name = alloc.memorylocations[0].name
        if alloc.kind == "ExternalInput":
            if name != partition_name:
                in_names.append(name)
        elif alloc.kind == "ExternalOutput":
            shape = tuple(alloc.tensor_shape)
            dtype = mybir.dt.np(alloc.dtype)
            out_names.append(name)
            out_avals.append(jax.core.ShapedArray(shape, dtype))
    n_params = len(in_names)
    n_outs = len(out_avals)
    all_in_names = in_names + out_names
    if partition_name is not None:
        all_in_names.append(partition_name)
    donate = tuple(range(n_params, n_params + n_outs))

    def _body(*args):
        operands = list(args)
        if partition_name is not None:
            operands.append(b2j.partition_id_tensor())
        outs = b2j._bass_exec_p.bind(
            *operands,
            out_avals=tuple(out_avals),
            in_names=tuple(all_in_names),
            out_names=tuple(out_names),
            lowering_input_output_aliases=(),
            sim_require_finite=True,
            sim_require_nnan=True,
            nc=nc,
        )
        return tuple(outs)

    devices = jax.devices()[:N_CORES]
    mesh = Mesh(np.asarray(devices), ("core",))
    in_specs = (PartitionSpec("core"),) * (n_params + n_outs)
    out_specs = (PartitionSpec("core"),) * n_outs
    sharded = jax.jit(
        shard_map(_body, mesh=mesh, in_specs=in_specs, out_specs=out_specs,
                  check_rep=False),
        donate_argnums=donate,
        keep_unused=True,
    )

    zero_out_shapes = [
        ((N_CORES * a.shape[0], *a.shape[1:]), a.dtype) for a in out_avals
    ]

    def run(in_maps: list[dict[str, np.ndarray]]) -> list[dict[str, np.ndarray]]:
        concat_in = [
            np.concatenate([np.asarray(m[name]) for m in in_maps], axis=0)
            for name in in_names
        ]
        concat_zeros = [np.zeros(s, d) for s, d in zero_out_shapes]
        out_arrs = sharded(*concat_in, *concat_zeros)
        return [
            {
                name: np.asarray(out_arrs[i]).reshape(N_CORES, *out_avals[i].shape)[c]
                for i, name in enumerate(out_names)
            }
            for c in range(N_CORES)
        ]

    _CACHE["exec"] = run
    _CACHE["nc"] = nc
    return run


def _pack_weights(weights: np.ndarray) -> np.ndarray:
    """[o, c, kh, kw] f32 -> packed DR plan [c, oc_half, m, pair_half, o_local] fp8."""
    wf = np.asarray(weights, dtype=np.float32)
    w8 = wf.astype(E4)
    wlo = (wf - w8.astype(np.float32)).astype(E4)
    # [o, c, k] -> [c, k, o]
    w8t = w8.reshape(O, C, 9).transpose(1, 2, 0)
    wlot = wlo.reshape(O, C, 9).transpose(1, 2, 0)
    wpk = np.zeros((C, 2, 13, 2, 128), dtype=E4)
    for oc in range(2):
        sl = slice(oc * 128, (oc + 1) * 128)
        for m, (kind, a, bb) in enumerate(PLAN):
            if kind == "hilo":
                wpk[:, oc, m, 0, :] = w8t[:, a, sl]
                wpk[:, oc, m, 1, :] = w8t[:, a, sl]
            elif kind == "bcast":
                wpk[:, oc, m, 0, :] = w8t[:, a, sl]
                wpk[:, oc, m, 1, :] = wlot[:, a, sl]
            else:
                wpk[:, oc, m, 0, :] = wlot[:, a, sl]
                wpk[:, oc, m, 1, :] = wlot[:, bb, sl]
    return wpk


def kernel(x: np.ndarray, weights: np.ndarray) -> np.ndarray:
    xf = np.ascontiguousarray(x, dtype=np.float32)
    x_hi = xf.astype(E4)
    x_lo = (xf - x_hi.astype(np.float32)).astype(E4)
    # [b, c, 2, h, w]
    x8 = np.ascontiguousarray(np.stack([x_hi, x_lo], axis=2))
    wpk = _pack_weights(weights)

    run = _get_executor()
    in_maps = [
        {"x8": x8[i * B_LOC : (i + 1) * B_LOC], "wpk": wpk} for i in range(N_CORES)
    ]
    results = run(in_maps)
    out16 = np.concatenate([r["out"] for r in results], axis=0)
    return out16.astype(np.float32)
